# revision 31
# baseline (speedup 1.0000x reference)
"""Trainium2 Bass kernel for a Swin-style transformer block (nn_Block_53979148976597).

Data-parallel over batch B=8 across 8 NeuronCores (one element per core, no
collectives).  Two builds:

- fp8 fast path (used when LN weights are trivial, proj/fc2 biases are zero):
  all GEMMs run as fp8e4m3 DoubleRow matmuls (2 k-tiles, half-rate rows) with
  x64 host weight scaling undone at PSUM evacuation; the relative-position
  bias is added into the QK psum by an identity DoubleRow matmul; softmax exp
  is split between the Activation engine (true exp -> fp8) and the DVE
  (Schraudolph exponent bit-trick via uint8 bitcast); AV runs ea-stationary
  fp8 DoubleRow with softmax denominators from per-column ones matmuls; the
  residual spine stays exact f32.

- bf16 fallback (general): LN1 -> QKV -> attention with multiplicative
  exp(rpb) -> proj -> residual -> LN2 -> MLP with bf16 operands.
"""

import sys

sys.path.insert(0, "/opt/trn_rl_repo")

import numpy as np
import ml_dtypes

import concourse.bass as bass
import concourse.mybir as mybir
import concourse.tile as tile
from concourse import bacc
from concourse.bass_utils import run_bass_kernel_spmd
from concourse.masks import make_identity

F32 = mybir.dt.float32
BF16 = mybir.dt.bfloat16
F8 = mybir.dt.float8e4
U8 = mybir.dt.uint8
AF = mybir.ActivationFunctionType
OP = mybir.AluOpType
PM = mybir.MatmulPerfMode
F8NP = ml_dtypes.float8_e4m3

B, HH, WW, D = 8, 32, 32, 768
N = HH * WW
NH, HD = 12, 64
HID = 3072
EPS = 1e-5
SCALE = HD ** -0.5
NT = N // 128
FC = D // 128
HC = HID // 128
BN_SUB = 2
WS = 64.0                      # host weight scale for fp8 headroom
LOG2E = 1.4426950408889634
SCH_A = 8.0 * LOG2E
SCH_B = 56.5                   # e4m3 exponent offset + round-to-nearest

# engine split of the 96 softmax-exp chunks: 'A' true exp on Activation,
# 'D'/'P' Schraudolph bit-trick on DVE / GPSIMD.  Tuned so Act/DVE/Pool
# land at similar busy time.
def _exp_pattern(na, nd, np_):
    tot = na + nd + np_
    out, acc = [], {"A": 0.0, "D": 0.0, "P": 0.0}
    w = {"A": na / tot, "D": nd / tot, "P": np_ / tot}
    for _ in range(tot):
        for k in acc:
            acc[k] += w[k]
        pick = max(acc, key=lambda k: acc[k])
        acc[pick] -= 1.0
        out.append(pick)
    return out

EXP_ENG = _exp_pattern(56, 40, 0)    # len 96, applied as EXP_ENG[h * 8 + mc]
                                     # (GPSIMD cannot read PSUM on TRN2)


def _build_fp8(s1, s2):
    """s1/s2: host fp8 scaling of the gamma-folded proj/fc2 weights; their
    inverses are baked into the PSUM-evacuation scalars."""
    nc = bacc.Bacc("TRN2", target_bir_lowering=False, debug=False, enable_asserts=False)

    d_x = nc.dram_tensor("x_in", [N, D], F32, kind="ExternalInput").ap()
    d_wq = nc.dram_tensor("wq_in", [128, 3, FC, 2, 128], F8, kind="ExternalInput").ap()
    d_wk = nc.dram_tensor("wk_in", [128, 3, FC, 2, 128], F8, kind="ExternalInput").ap()
    d_wv = nc.dram_tensor("wv_in", [128, 3, 2, D], F8, kind="ExternalInput").ap()
    d_wp = nc.dram_tensor("wp_in", [128, 3, 2, D], F8, kind="ExternalInput").ap()
    d_f1 = nc.dram_tensor("f1_in", [HC, 128, 3, 2, 128], F8, kind="ExternalInput").ap()
    d_f2 = nc.dram_tensor("f2_in", [128, 12, 2, D], F8, kind="ExternalInput").ap()
    d_qb = nc.dram_tensor("qb_in", [128, FC], F32, kind="ExternalInput").ap()
    d_f1b = nc.dram_tensor("f1b_in", [128, HC], F32, kind="ExternalInput").ap()
    d_vb = nc.dram_tensor("vb_in", [1, D], F8, kind="ExternalInput").ap()
    d_rpb = nc.dram_tensor("rpb_in", [NH, 128, NT, N], F8, kind="ExternalInput").ap()
    d_out = nc.dram_tensor("y_out", [N, D], F32, kind="ExternalOutput").ap()

    def evac(eng, out, in_, scale=1.0, bias=None):
        """PSUM evacuation with optional scale (const) and per-partition bias
        AP, on a selectable engine."""
        if eng == "A":
            if bias is not None:
                nc.scalar.activation(out=out, in_=in_, func=AF.Identity,
                                     bias=bias, scale=scale)
            else:
                nc.scalar.activation(out=out, in_=in_, func=AF.Copy, scale=scale)
        else:
            e = nc.vector if eng == "D" else nc.gpsimd
            if bias is not None:
                e.tensor_scalar(out=out, in0=in_, scalar1=scale, scalar2=bias,
                                op0=OP.mult, op1=OP.add)
            elif scale != 1.0:
                e.tensor_scalar(out=out, in0=in_, scalar1=scale, scalar2=None,
                                op0=OP.mult)
            else:
                e.tensor_scalar(out=out, in0=in_, scalar1=0.0, scalar2=None,
                                op0=OP.add)

    with tile.TileContext(nc) as tc:
        with (
            tc.tile_pool(name="persist", bufs=1) as pp,
            tc.tile_pool(name="lnstat", bufs=8) as lsp,
        ):
            x_t = [pp.tile([128, D], F32, tag=f"x{i}", name=f"x{i}") for i in range(NT)]
            qb_t = pp.tile([128, FC], F32)
            f1b_t = pp.tile([128, HC], F32)
            vb_t = pp.tile([1, D], F8)
            ones1 = pp.tile([1, 128], F8)
            ident_bf = pp.tile([128, 128], BF16)
            eps_t = pp.tile([128, 1], F32)
            wq_t = pp.tile([128, 3, FC, 2, 128], F8)
            wk_t = pp.tile([128, 3, FC, 2, 128], F8)
            wv_t = pp.tile([128, 3, 2, D], F8)
            wp_t = pp.tile([128, 3, 2, D], F8)
            f2_t = pp.tile([128, 12, 2, D], F8)

            nc.gpsimd.memset(eps_t, EPS)
            nc.gpsimd.memset(ones1, 1.0)
            make_identity(nc, ident_bf)
            for i in range(4):
                nc.sync.dma_start(x_t[i], d_x[i * 128:(i + 1) * 128, :])
            nc.sync.dma_start(wq_t, d_wq)
            nc.sync.dma_start(wk_t, d_wk)
            for i in range(4, NT):
                nc.sync.dma_start(x_t[i], d_x[i * 128:(i + 1) * 128, :])
            nc.sync.dma_start(qb_t, d_qb)
            nc.sync.dma_start(wv_t, d_wv)
            nc.sync.dma_start(f1b_t, d_f1b)
            nc.sync.dma_start(vb_t, d_vb)
            # wp / f2 weight DMAs are deferred into the attention span so the
            # startup DMA burst doesn't gate the first QKV matmuls.

            def layernorm_fm(dst_fm, tr_pool, hat_pool, ntag, tr_eng, hat_eng="P"):
                """x_t (f32 token-major) -> x_hat bf16 -> PE transpose ->
                dst_fm fp8 [128, FC, N] feature-major."""
                hats = []
                for i in range(NT):
                    st = lsp.tile([128, BN_SUB, 6], F32, tag="bnst", name=f"bnst_{ntag}{i}")
                    mv = lsp.tile([128, 2], F32, tag="bnmv", name=f"bnmv_{ntag}{i}")
                    xv = x_t[i].rearrange("p (s f) -> p s f", s=BN_SUB)
                    for s in range(BN_SUB):
                        nc.vector.bn_stats(out=st[:, s, :], in_=xv[:, s, :])
                    nc.vector.bn_aggr(out=mv, in_=st)
                    sd = lsp.tile([128, 1], F32, tag="bnsd", name=f"bnsd_{ntag}{i}")
                    nc.scalar.activation(out=sd, in_=mv[:, 1:2], func=AF.Sqrt,
                                         bias=eps_t[:, 0:1], scale=1.0)
                    nc.vector.reciprocal(out=sd, in_=sd)
                    hat = hat_pool.tile([128, D], BF16, tag="xhat", bufs=8,
                                        name=f"xhat_{ntag}{i}")
                    if hat_eng[i % len(hat_eng)] == "P":
                        nc.gpsimd.tensor_scalar(out=hat, in0=x_t[i],
                                                scalar1=mv[:, 0:1], scalar2=sd[:, 0:1],
                                                op0=OP.subtract, op1=OP.mult)
                    else:
                        nb_t = lsp.tile([128, 1], F32, tag="bnnb", name=f"bnnb_{ntag}{i}")
                        nc.vector.scalar_tensor_tensor(out=nb_t, in0=mv[:, 0:1],
                                                       scalar=-1.0, in1=sd[:, 0:1],
                                                       op0=OP.mult, op1=OP.mult)
                        nc.scalar.activation(out=hat, in_=x_t[i], func=AF.Identity,
                                             bias=nb_t[:, 0:1], scale=sd[:, 0:1])
                    hats.append(hat)
                for half in range(2):
                    for c in range(FC):
                        ptr = tr_pool.tile([128, 512], BF16, tag="ptr",
                                           name=f"ptr_{ntag}{half}{c}")
                        for i4 in range(4):
                            i = half * 4 + i4
                            nc.tensor.transpose(ptr[:, i4 * 128:(i4 + 1) * 128],
                                                hats[i][:, c * 128:(c + 1) * 128],
                                                ident_bf)
                        evac(tr_eng[(half * FC + c) % len(tr_eng)],
                             dst_fm[:, c, half * 512:(half + 1) * 512], ptr)

            # ================= LN1 + QKV + attention =================
            with tc.tile_pool(name="attn_span", bufs=1) as asp:
                # q for head pair c: head 2c on partitions 0:64, 2c+1 on 64:128
                q_pair = asp.tile([128, FC, N], F8)
                zq = asp.tile([64, N], F8)
                # plane 0: identity blocks (repeated 8x); planes 1+c: K chunks
                k_all = asp.tile([128, 1 + FC, N], F8)
                v_sb = asp.tile([128, NT, NH * 65], F8)
                ao_sb = asp.tile([128, NT, D], BF16)
                ao_fm = asp.tile([128, FC, N], F8)
                va = v_sb.rearrange("p t (h j) -> p t h j", j=65)

                nc.gpsimd.memset(zq, 0.0)
                nc.gpsimd.memset(k_all[:, 0, :], 0.0)
                for blk in range(NT):
                    make_identity(nc, k_all[:, 0, blk * 128:(blk + 1) * 128],
                                  nomemset=True)
                nc.gpsimd.memset(va[:, :, :, 64:65], 1.0)

                with (
                    tc.tile_pool(name="ph12", bufs=5) as p12,
                    tc.tile_pool(name="ph3", bufs=3) as p3,
                    tc.tile_pool(name="ea_pool", bufs=2) as eap,
                ):
                    t_fm = p12.tile([128, FC, N], F8, tag="tfm", bufs=1, name="tfm")
                    with tc.tile_pool(name="ps_tr1", bufs=2, space="PSUM") as pst1:
                        layernorm_fm(t_fm, pst1, p12, "n1", tr_eng="AAAAAA",
                                     hat_eng="PA")

                    def rhs_pair(fm, kt, lo, sz):
                        return fm[:, 2 * kt:2 * kt + 2, lo:lo + sz]

                    with tc.tile_pool(name="ps_qk12", bufs=2, space="PSUM") as psq:
                        for nb in range(2):
                            for of in range(FC):
                                pq = psq.tile([128, 512], F32, tag="psq", name=f"psq{of}{nb}")
                                pk = psq.tile([128, 512], F32, tag="psk", name=f"psk{of}{nb}")
                                for kt in range(3):
                                    nc.tensor.matmul(pq, wq_t[:, kt, of],
                                                     rhs_pair(t_fm, kt, nb * 512, 512),
                                                     start=(kt == 0), stop=(kt == 2),
                                                     perf_mode=PM.DoubleRow)
                                for kt in range(3):
                                    nc.tensor.matmul(pk, wk_t[:, kt, of],
                                                     rhs_pair(t_fm, kt, nb * 512, 512),
                                                     start=(kt == 0), stop=(kt == 2),
                                                     perf_mode=PM.DoubleRow)
                                ns = slice(nb * 512, (nb + 1) * 512)
                                evac("A", q_pair[:, of, ns], pq,
                                     scale=1.0 / WS, bias=qb_t[:, of:of + 1])
                                evac("D", k_all[:, 1 + of, ns], pk, scale=1.0 / WS)

                    # ---- qrpb prefetch (DMA only; overlaps the V matmuls) ----
                    qrpb_ring = {}

                    def prefetch_qrpb(h):
                        c = h // 2
                        lv = slice(0, 64) if h % 2 == 0 else slice(64, 128)
                        dv = slice(64, 128) if h % 2 == 0 else slice(0, 64)
                        # planes 0..7: rpb per m-chunk, plane 8: q (DR partner)
                        qrpb = p3.tile([128, 9, N], F8, tag="qrpb", name=f"qrpb{h}")
                        nc.sync.dma_start(qrpb[:, 0:8, :], d_rpb[h])
                        nc.gpsimd.dma_start(qrpb[lv, 8, :], q_pair[lv, c, :])
                        nc.gpsimd.dma_start(qrpb[dv, 8, :], zq)
                        qrpb_ring[h] = qrpb

                    prefetch_qrpb(0)
                    prefetch_qrpb(1)
                    nc.sync.dma_start(wp_t, d_wp)

                    with tc.tile_pool(name="ps_v", bufs=2, space="PSUM") as psv_p:
                        for i in range(NT):
                            pv = psv_p.tile([128, 1024], F32, tag="psv", name=f"psv{i}")
                            for (off, sz) in ((0, 512), (512, 256)):
                                nc.tensor.matmul(pv[:, off:off + sz], ones1,
                                                 vb_t[:, off:off + sz],
                                                 start=True, stop=False, skip_group_check=True)
                                for kt in range(3):
                                    nc.tensor.matmul(pv[:, off:off + sz],
                                                     rhs_pair(t_fm, kt, i * 128, 128),
                                                     wv_t[:, kt, :, off:off + sz],
                                                     start=False, stop=(kt == 2),
                                                     skip_group_check=True,
                                                     perf_mode=PM.DoubleRow)
                            evac("A", va[:, i, :, 0:64], pv[:, 0:D], scale=1.0 / WS)

                    # ---------------- attention (software-pipelined) ----------------
                    with (
                        tc.tile_pool(name="ps_qk", bufs=3, space="PSUM") as psqk,
                        tc.tile_pool(name="ps_av", bufs=1, space="PSUM") as psav,
                    ):
                        ea_ring = {}
                        av_ring = {}

                        def emit_qk_exp(h, part):
                            c = h // 2
                            if part == 0:
                                qrpb_ring[h] = qrpb_ring.pop(h)
                                if h + 2 < NH:
                                    prefetch_qrpb(h + 2)
                                if h == 1:
                                    nc.sync.dma_start(f2_t, d_f2)
                                ea_ring[h] = eap.tile([128, NT, N], F8, tag="ea",
                                                      name=f"ea{h}")
                            qrpb = qrpb_ring[h]
                            ea_t = ea_ring[h]
                            for mc in range(part * 4, part * 4 + 4):
                                pqk = psqk.tile([128, 1024], F32, tag="pqk", name=f"pqk{h}{mc}")
                                lhs = k_all[:, 0:(c + 2):(c + 1), mc * 128:(mc + 1) * 128]
                                for nb in range(2):
                                    ns = slice(nb * 512, (nb + 1) * 512)
                                    nc.tensor.matmul(pqk[:, ns], lhs,
                                                     qrpb[:, mc:9:(8 - mc), ns],
                                                     start=True, stop=True,
                                                     skip_group_check=True,
                                                     perf_mode=PM.DoubleRow)
                                if EXP_ENG[h * 8 + mc] == "A":
                                    nc.scalar.activation(out=ea_t[:, mc, :], in_=pqk,
                                                         func=AF.Exp, scale=SCALE)
                                else:
                                    nc.vector.tensor_scalar(out=ea_t[:, mc, :].bitcast(U8),
                                                            in0=pqk, scalar1=SCH_A * SCALE,
                                                            scalar2=SCH_B,
                                                            op0=OP.mult, op1=OP.add)
                            if part == 1:
                                qrpb_ring.pop(h)

                        def emit_av(h, part):
                            # AV with denominator in column 64 of each 128-wide group
                            if part == 0:
                                av_ring[h] = psav.tile([128, 1024], F32, tag="av",
                                                       name=f"av{h}")
                            av = av_ring[h]
                            ea_t = ea_ring[h]
                            for nc_ in range(part * 4, part * 4 + 4):
                                ncs = slice(nc_ * 128, (nc_ + 1) * 128)
                                for i2 in range(4):
                                    nc.tensor.matmul(av[:, nc_ * 128:nc_ * 128 + 65],
                                                     ea_t[:, 2 * i2:2 * i2 + 2, ncs],
                                                     va[:, 2 * i2:2 * i2 + 2, h, :],
                                                     start=(i2 == 0), stop=(i2 == 3),
                                                     skip_group_check=True,
                                                     perf_mode=PM.DoubleRow)

                        def emit_rescale(h):
                            av = av_ring.pop(h)
                            ea_ring.pop(h)
                            avv = av.rearrange("p (g j) -> p g j", j=128)
                            rec = p3.tile([128, 8, 1], F32, tag="rec", name=f"rec{h}")
                            nc.vector.reciprocal(out=rec, in_=avv[:, :, 64:65])
                            nc.vector.tensor_tensor(
                                out=ao_sb[:, :, h * 64:h * 64 + 64],
                                in0=avv[:, :, 0:64],
                                in1=rec.broadcast_to([128, 8, 64]),
                                op=OP.mult)

                        for h in range(NH + 1):
                            if h < NH:
                                emit_qk_exp(h, 0)
                            if h >= 1:
                                emit_av(h - 1, 0)
                            if h < NH:
                                emit_qk_exp(h, 1)
                            if h >= 1:
                                emit_av(h - 1, 1)
                                emit_rescale(h - 1)

                # ------------- attn-out transpose + proj + residual -------------
                with (
                    tc.tile_pool(name="ps_tr3", bufs=2, space="PSUM") as pst3,
                    tc.tile_pool(name="ps_pj", bufs=2, space="PSUM") as pspj,
                ):
                    for cf in range(FC):
                        for half in range(2):
                            ptr = pst3.tile([128, 512], BF16, tag="ptr3",
                                            name=f"ptr3_{half}{cf}")
                            for i4 in range(4):
                                i = half * 4 + i4
                                nc.tensor.transpose(ptr[:, i4 * 128:(i4 + 1) * 128],
                                                    ao_sb[:, i, cf * 128:(cf + 1) * 128],
                                                    ident_bf)
                            evac("A", ao_fm[:, cf, half * 512:(half + 1) * 512], ptr)

                    for i in range(NT):
                        ppj = pspj.tile([128, 1024], F32, tag="ppj", name=f"ppj{i}")
                        for (off, sz) in ((0, 512), (512, 256)):
                            for kt in range(3):
                                nc.tensor.matmul(ppj[:, off:off + sz],
                                                 ao_fm[:, 2 * kt:2 * kt + 2, i * 128:(i + 1) * 128],
                                                 wp_t[:, kt, :, off:off + sz],
                                                 start=(kt == 0), stop=(kt == 2),
                                                 skip_group_check=True,
                                                 perf_mode=PM.DoubleRow)
                        nc.vector.scalar_tensor_tensor(
                            out=x_t[i], in0=ppj[:, 0:D],
                            scalar=1.0 / s1,
                            in1=x_t[i], op0=OP.mult, op1=OP.add)

            # ================= LN2 + MLP =================
            with (
                tc.tile_pool(name="ph4", bufs=5) as p4,
                tc.tile_pool(name="f1_ring", bufs=4) as f1r,
            ):
                m_fm = p4.tile([128, HC, N], F8, tag="mfm", bufs=1, name="mfm")
                t2_fm = p4.tile([128, FC, N], F8, tag="t2fm", bufs=1, name="t2fm")
                with tc.tile_pool(name="ps_tr2", bufs=2, space="PSUM") as pst2:
                    layernorm_fm(t2_fm, pst2, p4, "n2", tr_eng="AADAAD",
                                 hat_eng="AP")

                with (
                    tc.tile_pool(name="ps_f1", bufs=2, space="PSUM") as psf1,
                    tc.tile_pool(name="ps_f2", bufs=2, space="PSUM") as psf2,
                ):
                    for hc in range(HC):
                        f1w = f1r.tile([128, 3, 2, 128], F8, tag="f1w", name=f"f1w{hc}")
                        nc.gpsimd.dma_start(f1w, d_f1[hc])
                        pf1 = psf1.tile([128, 1024], F32, tag="pf1", name=f"pf1{hc}")
                        for nb in range(2):
                            for kt in range(3):
                                nc.tensor.matmul(pf1[:, nb * 512:(nb + 1) * 512], f1w[:, kt],
                                                 t2_fm[:, 2 * kt:2 * kt + 2, nb * 512:(nb + 1) * 512],
                                                 start=(kt == 0), stop=(kt == 2),
                                                 skip_group_check=True,
                                                 perf_mode=PM.DoubleRow)
                        nc.scalar.activation(out=m_fm[:, hc, :],
                                             in_=pf1, func=AF.Gelu,
                                             bias=f1b_t[:, hc:hc + 1], scale=1.0 / WS)

                    for i in range(NT):
                        y_sb = p4.tile([128, D], F32, tag="ysb", name=f"ysb{i}")
                        pf2 = psf2.tile([128, 1024], F32, tag="pf2", name=f"pf2{i}")
                        for (off, sz) in ((0, 512), (512, 256)):
                            for kt in range(12):
                                nc.tensor.matmul(pf2[:, off:off + sz],
                                                 m_fm[:, 2 * kt:2 * kt + 2, i * 128:(i + 1) * 128],
                                                 f2_t[:, kt, :, off:off + sz],
                                                 start=(kt == 0), stop=(kt == 11),
                                                 skip_group_check=True,
                                                 perf_mode=PM.DoubleRow)
                        nc.vector.scalar_tensor_tensor(
                            out=y_sb, in0=pf2[:, 0:D],
                            scalar=1.0 / s2,
                            in1=x_t[i], op0=OP.mult, op1=OP.add)
                        nc.gpsimd.dma_start(d_out[i * 128:(i + 1) * 128, :], y_sb)

    nc.compile()
    return nc


def prep_fp8_inputs(x1, qkv_w, q_bias, v_bias, proj_w, rel_pos_table, fc1_w,
                    fc1_b, fc2_w, gamma1, gamma2, rel_pos_index):
    """Host-side packing for the fp8 build. x1 is [N, D] f32 (one batch elem
    handled by caller); returns the shared input map (without x_in)."""
    f8 = lambda a: np.ascontiguousarray(a).astype(F8NP)

    def lhsT_pack(wT, oc):  # wT [768, oc*128] -> [128, 3, oc, 2, 128]
        return f8(wT.reshape(3, 2, 128, oc, 128).transpose(2, 0, 3, 1, 4) * WS)

    def rhs_pack(wT, nkt):  # wT [nkt*256, F] -> [128, nkt, 2, F]
        return f8(wT.reshape(nkt, 2, 128, wT.shape[1]).transpose(2, 0, 1, 3) * WS)

    wqT = qkv_w[:D].T
    wkT = qkv_w[D:2 * D].T
    wvT = qkv_w[2 * D:].T
    wp = proj_w.T * gamma1[None, :]
    f2 = fc2_w.T * gamma2[None, :]
    # normalize tiny gamma-folded weights into fp8 range, undone at evac
    # via the 1/WS scalar (gamma magnitude folded into WS-compensation).
    gm1 = max(np.max(np.abs(wp)), 1e-30)
    gm2 = max(np.max(np.abs(f2)), 1e-30)
    # power-of-two scales so the build cache key is stable for similar inputs
    s1 = float(2.0 ** np.floor(np.log2(8.0 / gm1)))
    s2 = float(2.0 ** np.floor(np.log2(8.0 / gm2)))

    rpb = rel_pos_table[np.asarray(rel_pos_index)]        # [n, m, NH]
    rpb = rpb.transpose(2, 1, 0) * (1.0 / SCALE)          # [NH, m, n], pre-scaled
    rpb = np.ascontiguousarray(rpb.reshape(NH, NT, 128, N).transpose(0, 2, 1, 3))

    return {
        "wq_in": lhsT_pack(wqT, FC),
        "wk_in": lhsT_pack(wkT, FC),
        "wv_in": rhs_pack(wvT, 3),
        "wp_in": f8(wp.reshape(3, 2, 128, D).transpose(2, 0, 1, 3) * s1),
        "f1_in": f8(fc1_w.T.reshape(3, 2, 128, HC, 128).transpose(3, 2, 0, 1, 4) * WS),
        "f2_in": f8(f2.reshape(12, 2, 128, D).transpose(2, 0, 1, 3) * s2),
        "qb_in": np.ascontiguousarray(q_bias.reshape(FC, 128).T),
        "f1b_in": np.ascontiguousarray(fc1_b.reshape(HC, 128).T),
        "vb_in": f8(v_bias.reshape(1, D) * WS),
        "rpb_in": f8(rpb),
    }, s1, s2


_BUILD_CACHE = {}
_PT_CACHE = {}
_LAST_IN_MAPS = None


def _build(trivial_norm1, trivial_norm2):

    nc = bacc.Bacc("TRN2", target_bir_lowering=False, debug=False, enable_asserts=False)

    d_x = nc.dram_tensor("x_in", [N, D], F32, kind="ExternalInput").ap()
    d_qkvw = nc.dram_tensor("qkvw_in", [3, D, D], BF16, kind="ExternalInput").ap()
    d_pw = nc.dram_tensor("pw_in", [D, D], BF16, kind="ExternalInput").ap()
    d_f1 = nc.dram_tensor("f1_in", [HC, FC, 128, 128], BF16, kind="ExternalInput").ap()
    d_f2 = nc.dram_tensor("f2_in", [HID, D], BF16, kind="ExternalInput").ap()
    d_qb = nc.dram_tensor("qb_in", [128, FC], F32, kind="ExternalInput").ap()
    d_vb = nc.dram_tensor("vb_in", [1, D], BF16, kind="ExternalInput").ap()
    d_f1b = nc.dram_tensor("f1b_in", [128, HC], F32, kind="ExternalInput").ap()
    d_pt = nc.dram_tensor("pt_in", [NH, NB, 128, NT, 512], BF16, kind="ExternalInput").ap()
    d_n1 = nc.dram_tensor("n1_in", [128, 2 * FC], F32, kind="ExternalInput").ap()
    d_n2 = nc.dram_tensor("n2_in", [128, 2 * FC], F32, kind="ExternalInput").ap()
    d_out = nc.dram_tensor("y_out", [N, D], F32, kind="ExternalOutput").ap()

    with tile.TileContext(nc) as tc:
        with (
            tc.tile_pool(name="persist", bufs=1) as pp,
            tc.tile_pool(name="bigw", bufs=1) as bwp,
            tc.tile_pool(name="lnstat", bufs=8) as lsp,
        ):
            # --- persistent small tiles -------------------------------------
            x_t = [pp.tile([128, D], F32, tag=f"x{i}", name=f"x{i}") for i in range(NT)]
            qb_t = pp.tile([128, FC], F32)
            f1b_t = pp.tile([128, HC], F32)
            vb_t = pp.tile([1, D], BF16)
            n1_t = pp.tile([128, 2 * FC], F32)
            n2_t = pp.tile([128, 2 * FC], F32)
            eps_t = pp.tile([128, 1], F32)
            ones_b = pp.tile([1, 128], BF16)     # K=1 v-bias matmul lhsT
            ones_f = pp.tile([65, 64], F32)      # K=1 denom broadcast lhsT (row 64)
            ao_fm = [pp.tile([128, N], BF16, tag=f"aofm{c}", name=f"aofm{c}") for c in range(FC)]
            pw_t = pp.tile([128, FC, D], BF16)
            ident_t = pp.tile([128, 128], BF16)
            nc.gpsimd.memset(eps_t, EPS)
            nc.gpsimd.memset(ones_b, 1.0)
            nc.gpsimd.memset(ones_f, 1.0)
            make_identity(nc, ident_t)
            for i in range(NT):
                nc.sync.dma_start(x_t[i], d_x[i * 128:(i + 1) * 128, :])
            nc.sync.dma_start(qb_t, d_qb)
            nc.sync.dma_start(f1b_t, d_f1b)
            nc.sync.dma_start(vb_t, d_vb)
            nc.sync.dma_start(n1_t, d_n1)
            nc.sync.dma_start(n2_t, d_n2)
            nc.sync.dma_start(pw_t, d_pw.rearrange("(c p) o -> p c o", p=128))

            # big-weight slot (recycled): qkv weights -> fc2 weights
            qkvw_t = bwp.tile([128, 3 * FC, D], BF16, tag="bigw", name="qkvw",
                              padded_shape=[128, HC, D])
            nc.sync.dma_start(qkvw_t, d_qkvw.rearrange("w (c p) o -> p (w c) o", p=128))

            def layernorm_to(dst_tiles, src_tiles, tr_pool, fm_tiles, ntag, norm_t, trivial):
                """src (token-major f32) -> x_hat bf16 -> PE transpose ->
                fm_tiles (feature-major bf16 [128, N]); per-feature w/b applied
                during the PSUM evacuation unless trivial."""
                for i in range(NT):
                    st = lsp.tile([128, BN_SUB, 6], F32, tag="bnst", name=f"bnst_{ntag}{i}")
                    mv = lsp.tile([128, 2], F32, tag="bnmv", name=f"bnmv_{ntag}{i}")
                    xv = src_tiles[i].rearrange("p (s f) -> p s f", s=BN_SUB)
                    for s in range(BN_SUB):
                        nc.vector.bn_stats(out=st[:, s, :], in_=xv[:, s, :])
                    nc.vector.bn_aggr(out=mv, in_=st)
                    sd = lsp.tile([128, 1], F32, tag="bnsd", name=f"bnsd_{ntag}{i}")
                    nc.scalar.activation(out=sd, in_=mv[:, 1:2], func=AF.Sqrt,
                                         bias=eps_t[:, 0:1], scale=1.0)
                    nc.vector.reciprocal(out=sd, in_=sd)
                    nc.vector.tensor_scalar(out=dst_tiles[i], in0=src_tiles[i],
                                            scalar1=mv[:, 0:1], scalar2=sd[:, 0:1],
                                            op0=OP.subtract, op1=OP.mult)
                    for c in range(FC):
                        ptr = tr_pool.tile([128, 128], BF16, tag="ptr", name=f"ptr_{ntag}{i}{c}")
                        nc.tensor.transpose(ptr, dst_tiles[i][:, c * 128:(c + 1) * 128], ident_t)
                        dst = fm_tiles[c][:, i * 128:(i + 1) * 128]
                        if trivial:
                            nc.scalar.activation(out=dst, in_=ptr, func=AF.Copy)
                        else:
                            nc.vector.tensor_scalar(out=dst, in0=ptr,
                                                    scalar1=norm_t[:, c:c + 1],
                                                    scalar2=norm_t[:, FC + c:FC + c + 1],
                                                    op0=OP.mult, op1=OP.add)

            # ============ phase 1+2: LN1, transpose, QKV ============
            with tc.tile_pool(name="attn_span", bufs=1) as asp:
                q_fm = [asp.tile([128, N], BF16, tag=f"qfm{c}", name=f"qfm{c}") for c in range(FC)]
                k_fm = [asp.tile([128, N], BF16, tag=f"kfm{c}", name=f"kfm{c}") for c in range(FC)]
                v_aug = [asp.tile([128, NH, 65], BF16, tag=f"vaug{i}", name=f"vaug{i}") for i in range(NT)]

                with (
                    tc.tile_pool(name="ph12", bufs=3) as p12,
                    tc.tile_pool(name="ps_qkv", bufs=2, space="PSUM") as psq,
                    tc.tile_pool(name="ps_tr1", bufs=2, space="PSUM") as pst1,
                ):
                    tpre = [p12.tile([128, D], BF16, tag="tpre", name=f"tpre{i}") for i in range(NT)]
                    t_fm = [p12.tile([128, N], BF16, tag=f"tfm{c}", bufs=1, name=f"tfm{c}") for c in range(FC)]
                    layernorm_to(tpre, x_t, pst1, t_fm, "n1", n1_t, trivial_norm1)

                    for i in range(NT):
                        nc.gpsimd.memset(v_aug[i], 1.0)

                    # Q / K feature-major: out [of_chunk 128, tokens]
                    for c in range(FC):
                        for nb in range(NB):
                            pq = psq.tile([128, 512], F32, tag="psq", name=f"psq{c}{nb}")
                            pk = psq.tile([128, 512], F32, tag="psk", name=f"psk{c}{nb}")
                            for ic in range(FC):
                                nc.tensor.matmul(pq, qkvw_t[:, ic, c * 128:(c + 1) * 128],
                                                 t_fm[ic][:, nb * 512:(nb + 1) * 512],
                                                 start=(ic == 0), stop=(ic == FC - 1))
                            nc.vector.tensor_scalar(out=q_fm[c][:, nb * 512:(nb + 1) * 512],
                                                    in0=pq, scalar1=qb_t[:, c:c + 1],
                                                    scalar2=None, op0=OP.add)
                            for ic in range(FC):
                                nc.tensor.matmul(pk, qkvw_t[:, FC + ic, c * 128:(c + 1) * 128],
                                                 t_fm[ic][:, nb * 512:(nb + 1) * 512],
                                                 start=(ic == 0), stop=(ic == FC - 1))
                            nc.scalar.activation(out=k_fm[c][:, nb * 512:(nb + 1) * 512],
                                                 in_=pk, func=AF.Copy)

                    # V token-major (weights as rhs), with v_bias via K=1 ones matmul
                    for i in range(NT):
                        for (off, sz, h0, nh) in ((0, 512, 0, 8), (512, 256, 8, 4)):
                            pv = psq.tile([128, 512], F32, tag="psv", name=f"psv{i}{off}")
                            nc.tensor.matmul(pv[:, 0:sz], ones_b, vb_t[:, off:off + sz],
                                             start=True, stop=False, skip_group_check=True)
                            for ic in range(FC):
                                nc.tensor.matmul(pv[:, 0:sz],
                                                 t_fm[ic][:, i * 128:(i + 1) * 128],
                                                 qkvw_t[:, 2 * FC + ic, off:off + sz],
                                                 start=False, stop=(ic == FC - 1),
                                                 skip_group_check=True)
                            nc.scalar.activation(out=v_aug[i][:, h0:h0 + nh, 0:64],
                                                 in_=pv[:, 0:sz], func=AF.Copy)

                # fc2 weights into the recycled qkv slot (DMA overlaps attention)
                f2w_t = bwp.tile([128, HC, D], BF16, tag="bigw", name="f2w")
                nc.sync.dma_start(f2w_t, d_f2.rearrange("(c p) o -> p c o", p=128))
                # fc1 weights streamed per-hid-chunk (ring) during the MLP
                f1s = []
                

                # ============ phase 3: attention + proj ============
                with (
                    tc.tile_pool(name="ph3", bufs=2) as p3,
                    tc.tile_pool(name="ps_qk", bufs=2, space="PSUM") as psqk,
                    tc.tile_pool(name="ps_ao", bufs=2, space="PSUM") as psao,
                    tc.tile_pool(name="ps_aux", bufs=2, space="PSUM") as psaux,
                ):
                    for hp in range(NH // 2):
                        c = hp
                        for nb in range(NB):
                            pt_e = p3.tile([128, NT, 512], BF16, tag="pt_e", name=f"pte{hp}{nb}")
                            pt_o = p3.tile([128, NT, 512], BF16, tag="pt_o", name=f"pto{hp}{nb}")
                            nc.sync.dma_start(pt_e, d_pt[2 * hp, nb])
                            nc.sync.dma_start(pt_o, d_pt[2 * hp + 1, nb])
                            ea_e = p3.tile([128, NT, 512], BF16, tag="ea_e", name=f"eae{hp}{nb}")
                            ea_o = p3.tile([128, NT, 512], BF16, tag="ea_o", name=f"eao{hp}{nb}")
                            pao_e = psao.tile([65, 512], F32, tag="pao", name=f"paoe{hp}{nb}")
                            pao_o = psao.tile([65, 512], F32, tag="pao", name=f"paoo{hp}{nb}")
                            qs = slice(nb * 512, (nb + 1) * 512)
                            for mp in range(NT // 2):        # m-chunk pairs
                                pqk_e = psqk.tile([128, 1024], F32, tag="pqk", name=f"pqke{hp}{nb}{mp}")
                                pqk_o = psqk.tile([128, 1024], F32, tag="pqk", name=f"pqko{hp}{nb}{mp}")
                                for j in range(2):
                                    mc = 2 * mp + j
                                    ms = slice(mc * 128, (mc + 1) * 128)
                                    # adjacent row-group pair -> concurrent on HW
                                    nc.tensor.matmul(pqk_e[:, j * 512:(j + 1) * 512],
                                                     k_fm[c][0:64, ms], q_fm[c][0:64, qs],
                                                     start=True, stop=True)
                                    nc.tensor.matmul(pqk_o[:, j * 512:(j + 1) * 512],
                                                     k_fm[c][64:128, ms], q_fm[c][64:128, qs],
                                                     start=True, stop=True)
                                sl = slice(2 * mp, 2 * mp + 2)
                                nc.scalar.activation(out=ea_e[:, sl, :], in_=pqk_e, func=AF.Exp)
                                nc.vector.tensor_mul(ea_e[:, sl, :], ea_e[:, sl, :], pt_e[:, sl, :])
                                nc.scalar.activation(out=ea_o[:, sl, :], in_=pqk_o, func=AF.Exp)
                                nc.vector.tensor_mul(ea_o[:, sl, :], ea_o[:, sl, :], pt_o[:, sl, :])
                            for mc in range(NT):
                                nc.tensor.matmul(pao_e, v_aug[mc][:, 2 * hp, :], ea_e[:, mc, :],
                                                 start=(mc == 0), stop=(mc == NT - 1))
                            for mc in range(NT):
                                nc.tensor.matmul(pao_o, v_aug[mc][:, 2 * hp + 1, :], ea_o[:, mc, :],
                                                 start=(mc == 0), stop=(mc == NT - 1))
                            for (pao, r0, sfx) in ((pao_e, 0, "e"), (pao_o, 64, "o")):
                                den = p3.tile([65, 512], F32, tag="den", name=f"den{sfx}{hp}{nb}")
                                nc.vector.tensor_copy(den[64:65, :], pao[64:65, :])
                                prb = psaux.tile([64, 512], F32, tag="paux", name=f"prb{sfx}{hp}{nb}",
                                                 padded_shape=[128, 512])
                                nc.tensor.matmul(prb, ones_f[64:65, :], den[64:65, :],
                                                 start=True, stop=True)
                                rb = p3.tile([64, 512], F32, tag="rb", name=f"rb{sfx}{hp}{nb}")
                                nc.vector.reciprocal_approx_fast(rb, prb)
                                nc.vector.tensor_mul(
                                    ao_fm[c][r0:r0 + 64, nb * 512:(nb + 1) * 512],
                                    pao[0:64, :], rb)

                    # proj (token-major out) + residual into x tiles (x -> x2)
                    for i in range(NT):
                        for (off, sz) in ((0, 512), (512, 256)):
                            ppj = psaux.tile([128, sz], F32, tag="paux", name=f"ppj{i}{off}",
                                             padded_shape=[128, 512])
                            for c in range(FC):
                                nc.tensor.matmul(ppj,
                                                 ao_fm[c][:, i * 128:(i + 1) * 128],
                                                 pw_t[:, c, off:off + sz],
                                                 start=(c == 0), stop=(c == FC - 1))
                            nc.vector.scalar_tensor_tensor(
                                out=x_t[i][:, off:off + sz], in0=ppj, scalar=0.0,
                                in1=x_t[i][:, off:off + sz], op0=OP.bypass, op1=OP.add)

            # ============ phase 4: LN2 + MLP ============
            with (
                tc.tile_pool(name="ph4", bufs=3) as p4,
                tc.tile_pool(name="ps_f1", bufs=4, space="PSUM") as psf1,
                tc.tile_pool(name="ps_f2", bufs=2, space="PSUM") as psf2,
                tc.tile_pool(name="ps_tr2", bufs=2, space="PSUM") as pst2,
            ):
                m_fm = [p4.tile([128, N], BF16, tag=f"mfm{hc}", bufs=1, name=f"mfm{hc}") for hc in range(HC)]
                t2pre = [p4.tile([128, D], BF16, tag="t2pre", name=f"t2pre{i}") for i in range(NT)]
                t2_fm = [p4.tile([128, N], BF16, tag=f"t2fm{c}", bufs=1, name=f"t2fm{c}") for c in range(FC)]
                layernorm_to(t2pre, x_t, pst2, t2_fm, "n2", n2_t, trivial_norm2)

                for hc in range(HC):
                    f1w_hc = p4.tile([128, FC, 128], BF16, tag="f1w", bufs=4, name=f"f1w{hc}")
                    nc.sync.dma_start(f1w_hc, d_f1[hc].rearrange("c p o -> p c o"))
                    for nb in range(NB):
                        pf1 = psf1.tile([128, 512], F32, tag="pf1", name=f"pf1{hc}{nb}")
                        for ic in range(FC):
                            nc.tensor.matmul(pf1, f1w_hc[:, ic, :],
                                             t2_fm[ic][:, nb * 512:(nb + 1) * 512],
                                             start=(ic == 0), stop=(ic == FC - 1))
                        nc.scalar.activation(out=m_fm[hc][:, nb * 512:(nb + 1) * 512],
                                             in_=pf1, func=AF.Gelu,
                                             bias=f1b_t[:, hc:hc + 1], scale=1.0)

                for i in range(NT):
                    out_sb = p4.tile([128, D], F32, tag="outsb", name=f"outsb{i}")
                    for (off, sz) in ((0, 512), (512, 256)):
                        pf2 = psf2.tile([128, 512], F32, tag="pf2", name=f"pf2{i}{off}")
                        for hc in range(HC):
                            nc.tensor.matmul(pf2[:, 0:sz],
                                             m_fm[hc][:, i * 128:(i + 1) * 128],
                                             f2w_t[:, hc, off:off + sz],
                                             start=(hc == 0), stop=(hc == HC - 1))
                        nc.vector.scalar_tensor_tensor(
                            out=out_sb[:, off:off + sz], in0=pf2[:, 0:sz], scalar=0.0,
                            in1=x_t[i][:, off:off + sz], op0=OP.bypass, op1=OP.add)
                    nc.sync.dma_start(d_out[i * 128:(i + 1) * 128, :], out_sb)

    nc.compile()
    return nc



def _prep_pt(rel_pos_table, rel_pos_index):
    key = (hash(rel_pos_table.tobytes()), hash(rel_pos_index.tobytes()))
    if key in _PT_CACHE:
        return _PT_CACHE[key]
    rpb = rel_pos_table[rel_pos_index]                 # [n, m, NH] f32
    pt = np.exp(rpb.transpose(2, 1, 0))                # [NH, m, n]
    # -> [NH, NB, 128, NT, 512]: tile (h, nb)[p, mc, f] = pt[h, mc*128+p, nb*512+f]
    pt = pt.reshape(NH, NT, 128, NB, 512).transpose(0, 3, 2, 1, 4)
    pt = np.ascontiguousarray(pt).astype(ml_dtypes.bfloat16)
    _PT_CACHE.clear()
    _PT_CACHE[key] = pt
    return pt



def _fp8_eligible(norm1_w, norm1_b, norm2_w, norm2_b, proj_b, fc2_b):
    return (np.all(norm1_w == 1.0) and np.all(norm1_b == 0.0)
            and np.all(norm2_w == 1.0) and np.all(norm2_b == 0.0)
            and np.all(proj_b == 0.0) and np.all(fc2_b == 0.0))


def kernel(x, norm1_w, norm1_b, qkv_w, q_bias, v_bias, proj_w, proj_b,
           rel_pos_table, norm2_w, norm2_b, fc1_w, fc1_b, fc2_w, fc2_b,
           gamma1, gamma2, rel_pos_index):
    x = np.asarray(x, np.float32)
    f32 = lambda a: np.asarray(a, np.float32)
    bf = lambda a: np.ascontiguousarray(a).astype(ml_dtypes.bfloat16)
    norm1_w, norm1_b = f32(norm1_w), f32(norm1_b)
    norm2_w, norm2_b = f32(norm2_w), f32(norm2_b)
    qkv_w, proj_w = f32(qkv_w), f32(proj_w)
    fc1_w, fc2_w = f32(fc1_w), f32(fc2_w)
    gamma1, gamma2 = f32(gamma1), f32(gamma2)
    proj_b, fc1_b, fc2_b = f32(proj_b), f32(fc1_b), f32(fc2_b)
    q_bias, v_bias = f32(q_bias), f32(v_bias)
    rel_pos_table_f = f32(rel_pos_table)

    global _LAST_IN_MAPS
    xr = x.reshape(B, N, D)

    if _fp8_eligible(norm1_w, norm1_b, norm2_w, norm2_b, proj_b, fc2_b):
        shared, s1, s2 = prep_fp8_inputs(
            None, qkv_w, q_bias, v_bias, proj_w, rel_pos_table_f, fc1_w,
            fc1_b, fc2_w, gamma1, gamma2, rel_pos_index)
        key = ("fp8", s1, s2)
        if key not in _BUILD_CACHE:
            _BUILD_CACHE[key] = _build_fp8(s1, s2)
        nc = _BUILD_CACHE[key]
        in_maps = [dict(shared, x_in=np.ascontiguousarray(xr[i])) for i in range(B)]
        _LAST_IN_MAPS = in_maps
        res = run_bass_kernel_spmd(nc, in_maps, list(range(B)))
        out = np.stack([res.results[i]["y_out"] for i in range(B)], axis=0)
        return out.reshape(B, HH, WW, D).astype(np.float32)

    # ---------------- bf16 fallback ----------------
    qkv_wT = qkv_w.T                                    # [D, 3D]
    qkvw = np.stack([qkv_wT[:, :D] * SCALE, qkv_wT[:, D:2 * D], qkv_wT[:, 2 * D:]], axis=0)
    qkvw = bf(qkvw)                                     # [3, D, D]
    pw = bf(proj_w.T * gamma1[None, :])
    f1w = bf(fc1_w.T.reshape(FC, 128, HC, 128).transpose(2, 0, 1, 3))
    f2w = bf(fc2_w.T * gamma2[None, :])
    qb = np.ascontiguousarray((q_bias * SCALE).reshape(FC, 128).T)
    f1b = np.ascontiguousarray(fc1_b.reshape(HC, 128).T)
    vb = bf(v_bias.reshape(1, D))
    n1 = np.ascontiguousarray(np.concatenate([norm1_w.reshape(FC, 128).T,
                                              norm1_b.reshape(FC, 128).T], axis=1))
    n2 = np.ascontiguousarray(np.concatenate([norm2_w.reshape(FC, 128).T,
                                              norm2_b.reshape(FC, 128).T], axis=1))
    pt = _prep_pt(rel_pos_table_f, np.asarray(rel_pos_index))

    assert np.all(proj_b * gamma1 == 0.0) and np.all(fc2_b * gamma2 == 0.0), \
        "nonzero proj_b/fc2_b not supported by this kernel build"

    trivial1 = bool(np.all(norm1_w == 1.0) and np.all(norm1_b == 0.0))
    trivial2 = bool(np.all(norm2_w == 1.0) and np.all(norm2_b == 0.0))
    key = (trivial1, trivial2)
    if key not in _BUILD_CACHE:
        _BUILD_CACHE[key] = _build(trivial1, trivial2)
    nc = _BUILD_CACHE[key]

    shared = {
        "qkvw_in": qkvw, "pw_in": pw,
        "f1_in": f1w, "f2_in": f2w, "qb_in": qb, "vb_in": vb,
        "f1b_in": f1b, "pt_in": pt, "n1_in": n1, "n2_in": n2,
    }
    in_maps = [dict(shared, x_in=np.ascontiguousarray(xr[i])) for i in range(B)]
    _LAST_IN_MAPS = in_maps
    res = run_bass_kernel_spmd(nc, in_maps, list(range(B)))
    out = np.stack([res.results[i]["y_out"] for i in range(B)], axis=0)
    return out.reshape(B, HH, WW, D).astype(np.float32)

